# revision 1
# baseline (speedup 1.0000x reference)
"""Trainium2 Bass kernel for CudaMorphUnpool2D (max-unpool scatter + 3x3 dilation).

Strategy:
  - 1024 (b,c) planes sharded 128/core across 8 NeuronCores (fully data parallel).
  - Per core, the 128 planes sit on the 128 SBUF partitions; spatial dims live on
    the free axis so all window shifts are plain free-dim AP offsets.
  - Host prep: d = provenance - (2i*W + 2j) in {0,1,2,256,257,258,512,513,514}
    encodes (dy,dx) of each pooled cell's scatter target.  The scatter canvas is
    built as 4 parity-quadrant grids via compare+select chains that reproduce the
    reference's last-writer-wins scatter order, then a separable 3-tap max.
  - fp16 pipeline (values exactly representable / tiny rounding; doubles DVE rate
    and halves DMA traffic).  Set DT="float32" for a bit-exact (slower) pipeline.
"""
import os
import sys
import numpy as np
from contextlib import ExitStack

H, W = 256, 256
HP, WP = 128, 128
SI = 16                 # pooled rows per slab
NSLAB = HP // SI
NCORES = 8
PPC = 128               # planes per core

DT = os.environ.get("MORPH_DT", "float16")

for _p in ("/opt/trn_rl_repo", "/root/.axon_site/_ro/trn_rl_repo"):
    if os.path.isdir(_p) and _p not in sys.path:
        sys.path.append(_p)


def _build_nc(dt_name):
    import concourse.bass as bass  # noqa: F401
    import concourse.tile as tile
    from concourse import bacc, mybir

    dt = getattr(mybir.dt, dt_name)
    mdt = mybir.dt.uint16 if dt_name == "float16" else mybir.dt.int32
    AO = mybir.AluOpType

    nc = bacc.Bacc("TRN2", target_bir_lowering=False, debug=False)
    d_in = nc.dram_tensor("d", [PPC, HP, WP], dt, kind="ExternalInput").ap()
    f_in = nc.dram_tensor("f", [PPC, HP, WP], dt, kind="ExternalInput").ap()
    o_out = nc.dram_tensor("out", [PPC, H, W], dt, kind="ExternalOutput").ap()

    with tile.TileContext(nc) as tc, ExitStack() as ctx:
        pin = ctx.enter_context(tc.tile_pool(name="pin", bufs=2))
        pv = ctx.enter_context(tc.tile_pool(name="pv", bufs=1))
        pm = ctx.enter_context(tc.tile_pool(name="pm", bufs=1))
        pcm = ctx.enter_context(tc.tile_pool(name="pcm", bufs=2))
        pcq = ctx.enter_context(tc.tile_pool(name="pcq", bufs=1))
        pq = ctx.enter_context(tc.tile_pool(name="pq", bufs=1))
        pout = ctx.enter_context(tc.tile_pool(name="pout", bufs=2))

        for s in range(NSLAB):
            i0 = s * SI
            # --- input tiles: rows h in [0,18) <-> pooled row i0-1+h; cols 0,1 guard, 2+b
            D = pin.tile([128, SI + 2, 130], dt, tag="D")
            F = pin.tile([128, SI + 2, 130], dt, tag="F")
            rlo = max(0, i0 - 1)
            rhi = min(HP, i0 + SI + 1)
            hlo = rlo - (i0 - 1)
            hhi = rhi - (i0 - 1)
            nc.gpsimd.memset(D[:, :, 0:2], 0.0)
            nc.gpsimd.memset(F[:, :, 0:2], 0.0)
            if hlo > 0:
                nc.gpsimd.memset(D[:, 0:hlo, :], 0.0)
                nc.gpsimd.memset(F[:, 0:hlo, :], 0.0)
            if hhi < SI + 2:
                nc.gpsimd.memset(D[:, hhi:, :], 0.0)
                nc.gpsimd.memset(F[:, hhi:, :], 0.0)
            nc.sync.dma_start(D[:, hlo:hhi, 2:130], d_in[:, rlo:rhi, :])
            nc.sync.dma_start(F[:, hlo:hhi, 2:130], f_in[:, rlo:rhi, :])

            # --- quadrant canvas grids
            # E-grids (even cols): interior [0:128), guard cols 128,129
            # O-grids (odd cols):  guard cols 0,1, interior [2:130)
            V_ee = pv.tile([128, 17, 130], dt, tag="V_ee")
            V_oe = pv.tile([128, 17, 130], dt, tag="V_oe")
            V_eo = pv.tile([128, 17, 130], dt, tag="V_eo")
            V_oo = pv.tile([128, 17, 130], dt, tag="V_oo")
            nc.gpsimd.memset(V_ee[:, :, 128:130], 0.0)
            nc.gpsimd.memset(V_oe[:, :, 128:130], 0.0)
            nc.gpsimd.memset(V_eo[:, :, 0:2], 0.0)
            nc.gpsimd.memset(V_oo[:, :, 0:2], 0.0)

            # V_oo[a,b] = (D[a,b]==257)*F[a,b]            rows a=i0-1+h, h=0:17
            mv1 = pm.tile([128, 17, 130], dt, tag="mv1")
            nc.vector.tensor_scalar(mv1[:, :, 0:128], D[:, 0:17, 2:130], 257.0, None, AO.is_equal)
            nc.vector.tensor_tensor(V_oo[:, :, 2:130], mv1[:, :, 0:128], F[:, 0:17, 2:130], AO.mult)
            # V_oe: lo (a,b-1)=258, hi (a,b)=256
            nc.vector.scalar_tensor_tensor(
                V_oe[:, :, 0:128], D[:, 0:17, 1:129], 258.0, F[:, 0:17, 1:129],
                AO.is_equal, AO.mult)
            m1 = pm.tile([128, 17, 130], mdt, tag="m1")
            nc.vector.tensor_scalar(m1[:, :, 0:128], D[:, 0:17, 2:130], 256.0, None, AO.is_equal)
            nc.vector.copy_predicated(V_oe[:, :, 0:128], m1[:, :, 0:128], F[:, 0:17, 2:130])
            # V_eo: lo (a-1,b)=513, hi (a,b)=1      rows a=i0+h, h=0:17
            mv2 = pm.tile([128, 17, 130], dt, tag="mv2")
            nc.vector.tensor_scalar(mv2[:, :, 0:128], D[:, 0:17, 2:130], 513.0, None, AO.is_equal)
            nc.vector.tensor_tensor(V_eo[:, :, 2:130], mv2[:, :, 0:128], F[:, 0:17, 2:130], AO.mult)
            m2 = pm.tile([128, 17, 130], mdt, tag="m2")
            nc.vector.tensor_scalar(m2[:, :, 0:128], D[:, 1:18, 2:130], 1.0, None, AO.is_equal)
            nc.vector.copy_predicated(V_eo[:, :, 2:130], m2[:, :, 0:128], F[:, 1:18, 2:130])
            # V_ee: (a-1,b-1)=514 -> (a-1,b)=512 -> (a,b-1)=2 -> (a,b)=0
            nc.vector.scalar_tensor_tensor(
                V_ee[:, :, 0:128], D[:, 0:17, 1:129], 514.0, F[:, 0:17, 1:129],
                AO.is_equal, AO.mult)
            m3 = pm.tile([128, 17, 130], mdt, tag="m3")
            nc.vector.tensor_scalar(m3[:, :, 0:128], D[:, 0:17, 2:130], 512.0, None, AO.is_equal)
            nc.vector.copy_predicated(V_ee[:, :, 0:128], m3[:, :, 0:128], F[:, 0:17, 2:130])
            m4 = pm.tile([128, 17, 130], mdt, tag="m4")
            nc.vector.tensor_scalar(m4[:, :, 0:128], D[:, 1:18, 1:129], 2.0, None, AO.is_equal)
            nc.vector.copy_predicated(V_ee[:, :, 0:128], m4[:, :, 0:128], F[:, 1:18, 1:129])
            m5 = pm.tile([128, 17, 130], mdt, tag="m5")
            nc.vector.tensor_scalar(m5[:, :, 0:128], D[:, 1:18, 2:130], 0.0, None, AO.is_equal)
            nc.vector.copy_predicated(V_ee[:, :, 0:128], m5[:, :, 0:128], F[:, 1:18, 2:130])

            # --- colmax: ACT makes 4B-aligned shifted V copies, DVE does 2x maxes
            # into quadrant-contiguous cm tiles, ACT interleaves into cm_e/cm_o.
            Ve_sh = pm.tile([128, 17, 128], dt, tag="Ve_sh")
            Vo_sh = pm.tile([128, 17, 128], dt, tag="Vo_sh")
            Voe_sh = pm.tile([128, 17, 128], dt, tag="Voe_sh")
            Voo_sh = pm.tile([128, 17, 128], dt, tag="Voo_sh")
            nc.scalar.copy(Ve_sh[:], V_ee[:, :, 1:129])
            nc.scalar.copy(Vo_sh[:], V_eo[:, :, 1:129])
            nc.scalar.copy(Voe_sh[:], V_oe[:, :, 1:129])
            nc.scalar.copy(Voo_sh[:], V_oo[:, :, 1:129])
            P_e = pm.tile([128, 17, 128], dt, tag="P_e")
            P_o = pm.tile([128, 17, 128], dt, tag="P_o")
            nc.vector.tensor_tensor(P_e[:], V_ee[:, :, 0:128], V_eo[:, :, 2:130], AO.max)
            nc.vector.tensor_tensor(P_o[:], V_oe[:, :, 0:128], V_oo[:, :, 2:130], AO.max)
            cm_eE = pcq.tile([128, 17, 128], dt, tag="cm_eE")
            cm_eO = pcq.tile([128, 17, 128], dt, tag="cm_eO")
            cm_oE = pcq.tile([128, 17, 128], dt, tag="cm_oE")
            cm_oO = pcq.tile([128, 17, 128], dt, tag="cm_oO")
            nc.vector.tensor_tensor(cm_eE[:], Vo_sh[:], P_e[:], AO.max)
            nc.vector.tensor_tensor(cm_eO[:], P_e[:], Ve_sh[:], AO.max)
            nc.vector.tensor_tensor(cm_oE[:], Voo_sh[:], P_o[:], AO.max)
            nc.vector.tensor_tensor(cm_oO[:], P_o[:], Voe_sh[:], AO.max)
            cm_e = pcm.tile([128, 17, 256], dt, tag="cm_e")
            cm_o = pcm.tile([128, 17, 256], dt, tag="cm_o")
            cm_e_v = cm_e[:].rearrange("p r (b two) -> p r b two", two=2)
            cm_o_v = cm_o[:].rearrange("p r (b two) -> p r b two", two=2)
            nc.scalar.copy(cm_e_v[:, :, :, 0], cm_eE[:])
            nc.scalar.copy(cm_e_v[:, :, :, 1], cm_eO[:])
            nc.scalar.copy(cm_o_v[:, :, :, 0], cm_oE[:])
            nc.scalar.copy(cm_o_v[:, :, :, 1], cm_oO[:])

            # --- rowmax: out rows [2*i0, 2*i0+32)
            out_t = pout.tile([128, 32, 256], dt, tag="out_t")
            Q = pq.tile([128, 16, 256], dt, tag="Q")
            out_v = out_t[:].rearrange("p (r two) c -> p r two c", two=2)
            nc.vector.tensor_tensor(Q[:], cm_e[:, 0:16, :], cm_o[:, 1:17, :], AO.max)
            nc.vector.tensor_tensor(out_v[:, :, 0, :], cm_o[:, 0:16, :], Q[:], AO.max)
            nc.vector.tensor_tensor(out_v[:, :, 1, :], Q[:], cm_e[:, 1:17, :], AO.max)

            nc.sync.dma_start(o_out[:, 2 * i0:2 * i0 + 32, :], out_t[:])

    nc.compile()
    return nc


_NC_CACHE = {}


def _get_nc():
    if DT not in _NC_CACHE:
        _NC_CACHE[DT] = _build_nc(DT)
    return _NC_CACHE[DT]


def kernel(**inputs):
    f = np.asarray(inputs["f"])
    p = np.asarray(inputs["provenance"])
    B, C = f.shape[:2]
    assert f.shape == (B, C, HP, WP) and B * C == NCORES * PPC

    np_dt = np.float16 if DT == "float16" else np.float32
    base = (np.arange(HP, dtype=np.int32)[:, None] * (2 * W)
            + np.arange(WP, dtype=np.int32)[None, :] * 2)
    d = (p.reshape(B * C, HP, WP) - base[None]).astype(np_dt)
    fv = np.ascontiguousarray(f.reshape(B * C, HP, WP).astype(np_dt))
    d = np.ascontiguousarray(d)

    nc = _get_nc()
    from concourse.bass_utils import run_bass_kernel_spmd
    in_maps = [{"d": d[k * PPC:(k + 1) * PPC], "f": fv[k * PPC:(k + 1) * PPC]}
               for k in range(NCORES)]
    res = run_bass_kernel_spmd(nc, in_maps, core_ids=list(range(NCORES)))
    out = np.concatenate([res.results[k]["out"] for k in range(NCORES)], axis=0)
    return out.reshape(B, C, H, W).astype(np.float32)



# revision 3
# speedup vs baseline: 1.4501x; 1.4501x over previous
"""Trainium2 Bass kernel for CudaMorphUnpool2D (max-unpool scatter + 3x3 dilation).

Strategy (v5):
  - 1024 (b,c) planes sharded 128/core across 8 NeuronCores (fully data parallel).
  - Host pre-computes, per pooled cell, its scatter target as a flat index into a
    chunked, parity-split canvas layout (64 chunks/plane, each chunk = 2 canvas
    pair-rows laid out as 4 quadrant segments [ee|oe|eo|oo] of 130 cols w/ guard
    slots).  Cells overwritten by a later raster writer (reference last-writer-
    wins semantics) get index -1 (skipped), so indices are collision-free.
  - Device: GPSIMD local_scatter builds the canvas (zeroing dst = free guards /
    empty cells), DVE does the separable 3x3 max (colmax + rowmax) with ACT
    making the 4B-aligned shifted copies, planar [E|O] column output.
  - Host post: deinterleave output columns, cast fp16 -> fp32.
"""
import os
import sys
import numpy as np
from contextlib import ExitStack

H, W = 256, 256
HP, WP = 128, 128
NCORES = 8
PPC = 128               # planes per core

NCHUNK = 64             # canvas chunks per plane (2 pair-rows each)
NIDX = 384              # source slots per chunk (3 pooled rows x 128)
SEG = 130               # quadrant segment width (128 + 2 guard slots)
PAIRW = 4 * SEG         # 520 elements per canvas pair-row
NELEM = 2 * PAIRW       # 1040 elements per chunk
NSLAB = 8               # 8 slabs x 8 chunks
CPS = NCHUNK // NSLAB   # chunks per slab = 8

for _p in ("/opt/trn_rl_repo", "/root/.axon_site/_ro/trn_rl_repo"):
    if os.path.isdir(_p) and _p not in sys.path:
        sys.path.append(_p)


def _build_nc():
    import concourse.bass as bass  # noqa: F401
    import concourse.tile as tile
    from concourse import bacc, mybir

    dt = mybir.dt.float16
    AO = mybir.AluOpType

    nc = bacc.Bacc("TRN2", target_bir_lowering=False, debug=False)
    f_in = nc.dram_tensor("f", [PPC, HP, WP], dt, kind="ExternalInput").ap()
    ix_in = nc.dram_tensor("ix", [PPC, NCHUNK, NIDX], mybir.dt.int16,
                           kind="ExternalInput").ap()
    o_out = nc.dram_tensor("out", [PPC, H, W], dt, kind="ExternalOutput").ap()

    with tile.TileContext(nc) as tc, ExitStack() as ctx:
        pf = ctx.enter_context(tc.tile_pool(name="pf", bufs=2))
        pix = ctx.enter_context(tc.tile_pool(name="pix", bufs=2))
        pcv = ctx.enter_context(tc.tile_pool(name="pcv", bufs=3))
        psh = ctx.enter_context(tc.tile_pool(name="psh", bufs=2))
        pcm = ctx.enter_context(tc.tile_pool(name="pcm", bufs=2))
        pp = ctx.enter_context(tc.tile_pool(name="pp", bufs=2))
        pout = ctx.enter_context(tc.tile_pool(name="pout", bufs=2))

        canvases = {}

        def scatter_slab(s):
            """local_scatter chunks 8s..8s+7 into one flat canvas tile."""
            F = pf.tile([128, 17, WP], dt, tag="F")
            IX = pix.tile([128, CPS, NIDX], mybir.dt.int16, tag="IX")
            # f rows: max(0, 2t-1) for t=8s (=16s-1), through 16s+15
            if s == 0:
                nc.gpsimd.memset(F[:, 0, :], 0.0)
                nc.sync.dma_start(F[:, 1:17, :], f_in[:, 0:16, :])
            else:
                nc.sync.dma_start(F[:], f_in[:, 16 * s - 1:16 * s + 16, :])
            nc.sync.dma_start(IX[:], ix_in[:, CPS * s:CPS * (s + 1), :])

            CV = pcv.tile([128, CPS * NELEM], dt, tag="CV")
            for c in range(CPS):
                t = CPS * s + c
                # local data rows: r0(t) - (16s - 1); only t=0 special (r0=0)
                lr = 1 if t == 0 else 2 * c
                data = F[:, lr:lr + 3, :].rearrange("p r c -> p (r c)")
                nc.gpsimd.local_scatter(
                    CV[:, c * NELEM:(c + 1) * NELEM], data, IX[:, c, :],
                    channels=128, num_elems=NELEM, num_idxs=NIDX)
            canvases[s] = CV
            return CV

        def dilate_slab(s):
            """Produce output rows [32s, 32s+32) from canvas tiles s-1, s, s+1."""
            CV = canvases[s]
            CVp = canvases.get(s - 1)
            CVn = canvases.get(s + 1)

            # quadrant views: [128, 16 pair-rows, 130] per seg
            def segs(cv):
                return cv[:].rearrange("p (a g w) -> p a g w", a=2 * CPS, g=4, w=SEG)

            v = segs(CV)
            vp = segs(CVp) if CVp is not None else None
            vn = segs(CVn) if CVn is not None else None

            # cm tiles: rows h=0..17 <-> canvas pair-row 16s-1+h
            cm_eE = pcm.tile([128, 18, 128], dt, tag="cm_eE")
            cm_eO = pcm.tile([128, 18, 128], dt, tag="cm_eO")
            cm_oE = pcm.tile([128, 18, 128], dt, tag="cm_oE")
            cm_oO = pcm.tile([128, 18, 128], dt, tag="cm_oO")
            P_e = pp.tile([128, 18, 128], dt, tag="P_e")
            P_o = pp.tile([128, 18, 128], dt, tag="P_o")
            sh_ee = psh.tile([128, 18, 128], dt, tag="sh_ee")
            sh_eo = psh.tile([128, 18, 128], dt, tag="sh_eo")
            sh_oe = psh.tile([128, 18, 128], dt, tag="sh_oe")
            sh_oo = psh.tile([128, 18, 128], dt, tag="sh_oo")

            def colmax(view, h0, h1, a0):
                """colmax pair-rows [a0, a0+(h1-h0)) of `view` into cm rows [h0,h1)."""
                n = h1 - h0
                a1 = a0 + n
                E_e = view[:, a0:a1, 0, 0:128]    # V_ee interior
                E_o = view[:, a0:a1, 1, 0:128]    # V_oe interior
                O_e = view[:, a0:a1, 2, 2:130]    # V_eo interior
                O_o = view[:, a0:a1, 3, 2:130]    # V_oo interior
                # ACT 4B-aligned shifted copies
                nc.scalar.copy(sh_ee[:, h0:h1, :], view[:, a0:a1, 0, 1:129])  # E_e(b+1)
                nc.scalar.copy(sh_oe[:, h0:h1, :], view[:, a0:a1, 1, 1:129])  # E_o(b+1)
                nc.scalar.copy(sh_eo[:, h0:h1, :], view[:, a0:a1, 2, 1:129])  # O_e(b-1)
                nc.scalar.copy(sh_oo[:, h0:h1, :], view[:, a0:a1, 3, 1:129])  # O_o(b-1)
                nc.vector.tensor_tensor(P_e[:, h0:h1, :], E_e, O_e, AO.max)
                nc.vector.tensor_tensor(P_o[:, h0:h1, :], E_o, O_o, AO.max)
                nc.vector.tensor_tensor(cm_eE[:, h0:h1, :], P_e[:, h0:h1, :], sh_eo[:, h0:h1, :], AO.max)
                nc.vector.tensor_tensor(cm_eO[:, h0:h1, :], P_e[:, h0:h1, :], sh_ee[:, h0:h1, :], AO.max)
                nc.vector.tensor_tensor(cm_oE[:, h0:h1, :], P_o[:, h0:h1, :], sh_oo[:, h0:h1, :], AO.max)
                nc.vector.tensor_tensor(cm_oO[:, h0:h1, :], P_o[:, h0:h1, :], sh_oe[:, h0:h1, :], AO.max)

            # halo above (cm row 0 <-> pair 16s-1 = prev tile local pair 15)
            if vp is None:
                for cm in (cm_eE, cm_eO, cm_oE, cm_oO):
                    nc.vector.memset(cm[:, 0, :], 0.0)
            else:
                colmax(vp, 0, 1, 2 * CPS - 1)
            # main rows (cm rows 1..16 <-> pairs 16s..16s+15)
            colmax(v, 1, 17, 0)
            # halo below (cm row 17 <-> pair 16s+16 = next tile local pair 0)
            if vn is None:
                for cm in (cm_eE, cm_eO, cm_oE, cm_oO):
                    nc.vector.memset(cm[:, 17, :], 0.0)
            else:
                colmax(vn, 17, 18, 0)

            # rowmax: out pair r (0..15): even row = max(cm_o[r], S[r]),
            # odd = max(S[r], cm_e[r+2]), S[r] = max(cm_e[r+1], cm_o[r+1])
            out_t = pout.tile([128, 16, 2, 2, 128], dt, tag="out_t")
            S_E = pp.tile([128, 16, 128], dt, tag="S_E")
            S_O = pp.tile([128, 16, 128], dt, tag="S_O")
            nc.vector.tensor_tensor(S_E[:], cm_eE[:, 1:17, :], cm_oE[:, 1:17, :], AO.max)
            nc.vector.tensor_tensor(S_O[:], cm_eO[:, 1:17, :], cm_oO[:, 1:17, :], AO.max)
            nc.vector.tensor_tensor(out_t[:, :, 0, 0, :], cm_oE[:, 0:16, :], S_E[:], AO.max)
            nc.vector.tensor_tensor(out_t[:, :, 0, 1, :], cm_oO[:, 0:16, :], S_O[:], AO.max)
            nc.vector.tensor_tensor(out_t[:, :, 1, 0, :], S_E[:], cm_eE[:, 2:18, :], AO.max)
            nc.vector.tensor_tensor(out_t[:, :, 1, 1, :], S_O[:], cm_eO[:, 2:18, :], AO.max)

            ov = o_out[:, 32 * s:32 * s + 32, :].rearrange(
                "p (r two) (x c) -> p r two x c", two=2, x=2)
            nc.sync.dma_start(ov, out_t[:])

        scatter_slab(0)
        scatter_slab(1)
        for s in range(NSLAB):
            if s + 2 < NSLAB:
                scatter_slab(s + 2)
            dilate_slab(s)

    nc.compile()
    return nc


_NC_CACHE = {}


def _get_nc():
    if "nc" not in _NC_CACHE:
        _NC_CACHE["nc"] = _build_nc()
    return _NC_CACHE["nc"]


def prepare_inputs(f, p):
    """Host prep: fp16 values + collision-free chunk-layout scatter indices.

    Returns (fv, idxs): fv [N,HP,WP] fp16, idxs [N,NCHUNK,NIDX] int16.
    """
    N = f.shape[0] * f.shape[1]
    f2 = f.reshape(N, HP, WP)
    p2 = p.reshape(N, HP, WP).astype(np.int32)

    base = (np.arange(HP, dtype=np.int32)[:, None] * (2 * W)
            + np.arange(WP, dtype=np.int32)[None, :] * 2)
    d = p2 - base[None]
    dy = d >> 8
    dx = d & 255

    def sh(a, di, dj):
        out = np.full_like(a, -9)
        si0, si1 = max(di, 0), HP + min(di, 0)
        sj0, sj1 = max(dj, 0), WP + min(dj, 0)
        out[:, si0 - di:si1 - di, sj0 - dj:sj1 - dj] = a[:, si0:si1, sj0:sj1]
        return out

    dyR, dxR = sh(dy, 0, 1), sh(dx, 0, 1)
    dyD0, dxD0 = sh(dy, 1, 0), sh(dx, 1, 0)
    dyDm, dxDm = sh(dy, 1, -1), sh(dx, 1, -1)
    dyDp, dxDp = sh(dy, 1, 1), sh(dx, 1, 1)
    killed = ((dx == 2) & (dxR == 0) & (dyR == dy)) | ((dy == 2) & (
        ((dx == 0) & (dyDm == 0) & (dxDm == 2)) |
        ((dyD0 == 0) & (dxD0 == dx)) |
        ((dx == 2) & (dyDp == 0) & (dxDp == 0))))

    y = 2 * np.arange(HP, dtype=np.int32)[None, :, None] + dy
    x = 2 * np.arange(WP, dtype=np.int32)[None, None, :] + dx
    a = y >> 1
    seg = (x & 1) * 2 + (y & 1)              # [ee, oe, eo, oo]
    local_idx = (a & 1) * PAIRW + seg * SEG + np.where(x & 1 == 1, 2, 0) + (x >> 1)
    chunk_of = a >> 1

    idxs = np.full((N, NCHUNK, NIDX), -1, dtype=np.int16)
    tt = np.arange(NCHUNK)
    r0 = np.maximum(0, 2 * tt - 1)           # first data row per chunk
    for rloc in range(3):
        iv = r0 + rloc                        # pooled row in data slot rloc
        li = local_idx[:, iv, :]
        co = chunk_of[:, iv, :]
        kk = killed[:, iv, :]
        idxs[:, :, rloc * 128:(rloc + 1) * 128] = np.where(
            (co == tt[None, :, None]) & ~kk, li, -1).astype(np.int16)

    fv = np.ascontiguousarray(f2.astype(np.float16))
    return fv, np.ascontiguousarray(idxs)


def postprocess(out_planar):
    """[K, 256, 256] planar rows [E|O] -> interleaved columns, fp32."""
    res = np.empty(out_planar.shape, dtype=np.float32)
    res[..., 0::2] = out_planar[..., 0:128]
    res[..., 1::2] = out_planar[..., 128:256]
    return res


def kernel(**inputs):
    f = np.asarray(inputs["f"])
    p = np.asarray(inputs["provenance"])
    B, C = f.shape[:2]
    assert f.shape == (B, C, HP, WP) and B * C == NCORES * PPC

    fv, idxs = prepare_inputs(f, p)

    nc = _get_nc()
    from concourse.bass_utils import run_bass_kernel_spmd
    in_maps = [{"f": fv[k * PPC:(k + 1) * PPC], "ix": idxs[k * PPC:(k + 1) * PPC]}
               for k in range(NCORES)]
    res = run_bass_kernel_spmd(nc, in_maps, core_ids=list(range(NCORES)))
    out = np.concatenate([postprocess(res.results[k]["out"]) for k in range(NCORES)],
                         axis=0)
    return out.reshape(B, C, H, W)
